# revision 1
# baseline (speedup 1.0000x reference)
"""EntropyGuidedAttention Trainium2 kernel.

B=2, N=2048, C=1024, H=16, Dh=64 on 8 NeuronCores:
data-parallel over batch (cores 0-3 -> batch 0, 4-7 -> batch 1), tensor-parallel
over heads within a batch group (4 heads per core). Each core computes its
heads' attention and a row-split partial of the output projection; the host
sums the 4 partials per batch.

Layouts (per core): x^T resident in SBUF; Q^T/K^T computed per head-pair
[128, N] (fp32r); the sigmoid gate (and the 1/sqrt(Dh) scale) is folded into
Q^T columns; scores are computed transposed S^T[m, nq] with two heads row-
packed in the PE array (K=64 each); exp runs on ACT over [128, 1024] PSUM
tiles; V carries an appended ones-column so the AV matmul also produces the
softmax row-sums; AV^T is normalized per head and feeds the output projection
as lhsT directly.
"""
import os
import sys

sys.path.insert(0, "/opt/trn_rl_repo")

import numpy as np

import concourse.bass as bass
import concourse.mybir as mybir
import concourse.tile as tile
from concourse import bacc
from concourse.bass_utils import run_bass_kernel_spmd

F32 = mybir.dt.float32
F32R = mybir.dt.float32r
EXP = mybir.ActivationFunctionType.Exp
SIGMOID = mybir.ActivationFunctionType.Sigmoid

B, N, C, H = 2, 2048, 1024, 16
DH = C // H          # 64
HPC = 4              # heads per core
PW = 2 * DH          # head-pair width = 128
P = 128
NCI = C // P         # 8 contraction chunks
NNB = 4              # nq blocks
NB = 512             # nq block size
NMI = N // P         # 16 m-chunks
SCALE = 1.0 / 8.0    # 1/sqrt(DH)

_CACHE = {}


def _r(ap):
    return ap.bitcast(F32R)


def _bcast_rows(nc, dst, row, nrows):
    """DMA-broadcast a [1, W] DRAM row across `nrows` SBUF partitions."""
    src = bass.AP(tensor=row.tensor, offset=row.offset,
                  ap=[[0, nrows]] + list(row.ap[1:]))
    nc.sync.dma_start(dst, src)


def _build(reps=1, tiny_out=False):
    nc = bacc.Bacc("TRN2", target_bir_lowering=False, debug=False, num_devices=8)

    xT = nc.dram_tensor("xT", [C, N], F32, kind="ExternalInput")
    wq = nc.dram_tensor("wq", [C, HPC * DH], F32, kind="ExternalInput")
    wk = nc.dram_tensor("wk", [C, HPC * DH], F32, kind="ExternalInput")
    wv = nc.dram_tensor("wv", [C, HPC * DH], F32, kind="ExternalInput")
    we = nc.dram_tensor("we", [C, HPC], F32, kind="ExternalInput")
    wo = nc.dram_tensor("wo", [HPC * DH, C], F32, kind="ExternalInput")
    ones64 = nc.dram_tensor("ones64", [P, NMI * HPC], F32, kind="ExternalInput")
    if tiny_out:
        outp_t = nc.dram_tensor("tiny", [P, 512], F32, kind="ExternalOutput")
    else:
        outp = nc.dram_tensor("outp", [N, C], F32, kind="ExternalOutput")

    with tile.TileContext(nc) as tc, (
        tc.tile_pool(name="big", bufs=1)) as big, (
        tc.tile_pool(name="roll", bufs=3)) as roll, (
        tc.tile_pool(name="roll2", bufs=2)) as roll2, (
        tc.tile_pool(name="espool", bufs=4)) as espool, (
        tc.tile_pool(name="dram", bufs=1, space="DRAM")) as dram:
        if tiny_out:
            outp = dram.tile([N, C], F32, tag="outp_int", name="outp_int")
        for rep in range(reps):
            # ---- resident SBUF inputs (chunked DMAs -> parallel queues) ----
            wes = big.tile([P, NCI, HPC], F32R, tag="wes", name=f"wes{rep}")
            nc.sync.dma_start(wes[:], we.rearrange("(o p) f -> p o f", p=P).bitcast(F32R))
            xs = big.tile([P, NCI, N], F32R, tag="xs", name=f"xs{rep}")
            xTv = xT.rearrange("(o p) n -> p o n", p=P).bitcast(F32R)
            for ci in range(2):
                nc.sync.dma_start(xs[:, ci, :], xTv[:, ci, :])
            wqs = big.tile([P, NCI, HPC * DH], F32R, tag="wqs", name=f"wqs{rep}")
            nc.sync.dma_start(wqs[:], wq.rearrange("(o p) f -> p o f", p=P).bitcast(F32R))
            wks = big.tile([P, NCI, HPC * DH], F32R, tag="wks", name=f"wks{rep}")
            nc.sync.dma_start(wks[:], wk.rearrange("(o p) f -> p o f", p=P).bitcast(F32R))
            for ci in range(2, NCI):
                nc.sync.dma_start(xs[:, ci, :], xTv[:, ci, :])
            wvs = big.tile([P, NCI, HPC * DH], F32R, tag="wvs", name=f"wvs{rep}")
            nc.sync.dma_start(wvs[:], wv.rearrange("(o p) f -> p o f", p=P).bitcast(F32R))
            wos = big.tile([P, 2, C], F32R, tag="wos", name=f"wos{rep}")
            nc.sync.dma_start(wos[:], wo.rearrange("(o p) f -> p o f", p=P).bitcast(F32R))

            QT = [big.tile([P, N], F32R, tag=f"qt{p}", name=f"qt{p}_{rep}")
                  for p in range(2)]
            KT = [big.tile([P, N], F32R, tag=f"kt{p}", name=f"kt{p}_{rep}")
                  for p in range(2)]
            Vn = big.tile([P, NMI, HPC, DH + 1], F32R, tag="vn", name=f"vn{rep}")
            E4 = big.tile([HPC, N], F32, tag="e4", name=f"e4{rep}")
            AVn = [big.tile([P, N], F32R, tag=f"avn{p}", name=f"avn{p}_{rep}")
                   for p in range(2)]
            estg = dram.tile([HPC, N], F32, tag="estg", name=f"estg{rep}")

            nc.sync.dma_start(
                Vn[:, :, :, DH:DH + 1],
                ones64[:].rearrange("p (m h) -> p m h", h=HPC)[:, :, :, None]
                .bitcast(F32R))
            onesrow = big.tile([1, DH], F32R, tag="onesrow", name=f"onesrow{rep}")
            nc.sync.dma_start(onesrow[:], ones64[0:1, 0:DH].bitcast(F32R))

            # ---- phase 1: projections -------------------------------------
            with tc.tile_pool(name=f"ps1_{rep}", bufs=2, space="PSUM") as ps1:
                # gate logits -> sigmoid -> *1/8 -> DRAM staging for broadcast
                for ib in range(NNB):
                    nq = slice(ib * NB, (ib + 1) * NB)
                    pe = ps1.tile([HPC, NB], F32, tag="p1", name=f"pe{rep}_{ib}")
                    for ci in range(NCI):
                        nc.tensor.matmul(pe[:], wes[:, ci, :], xs[:, ci, nq],
                                         start=(ci == 0), stop=(ci == NCI - 1))
                    nc.scalar.activation(E4[:, nq], pe[:], SIGMOID)
                    nc.vector.tensor_scalar_mul(E4[:, nq], E4[:, nq], SCALE)
                    nc.sync.dma_start(estg[:, nq], E4[:, nq])

                def k_group(pair, ib):
                    nq = slice(ib * NB, (ib + 1) * NB)
                    pk = ps1.tile([P, NB], F32, tag="p1", name=f"pk{rep}_{pair}_{ib}")
                    for ci in range(NCI):
                        nc.tensor.matmul(
                            pk[:], wks[:, ci, pair * PW:(pair + 1) * PW],
                            xs[:, ci, nq],
                            start=(ci == 0), stop=(ci == NCI - 1))
                    nc.vector.tensor_copy(KT[pair][:, nq], pk[:])

                def q_group(pair, ib):
                    nq = slice(ib * NB, (ib + 1) * NB)
                    pq = ps1.tile([P, NB], F32, tag="p1", name=f"pq{rep}_{pair}_{ib}")
                    for ci in range(NCI):
                        nc.tensor.matmul(
                            pq[:], wqs[:, ci, pair * PW:(pair + 1) * PW],
                            xs[:, ci, nq],
                            start=(ci == 0), stop=(ci == NCI - 1))
                    g = roll2.tile([P, NB], F32, tag="g")
                    for half in range(2):
                        _bcast_rows(nc, g[half * DH:(half + 1) * DH, :],
                                    estg[2 * pair + half:2 * pair + half + 1, nq],
                                    DH)
                    nc.vector.tensor_mul(QT[pair][:, nq], pq[:], g[:])

                def v_group(mi):
                    pv = ps1.tile([P, HPC * DH], F32, tag="p1", name=f"pv{rep}_{mi}")
                    for ci in range(NCI):
                        nc.tensor.matmul(pv[:], xs[:, ci, mi * P:(mi + 1) * P],
                                         wvs[:, ci, :],
                                         start=(ci == 0), stop=(ci == NCI - 1))
                    nc.vector.tensor_copy(Vn[:, mi, :, 0:DH],
                                          pv[:].rearrange("p (h d) -> p h d", h=HPC))

                # pair 0 first so attention can start while pair 1 projects
                for ib in range(NNB):
                    k_group(0, ib)
                for ib in range(NNB):
                    q_group(0, ib)

                # ---- phase 2/3: attention (pair-major; overlaps pair-1
                # projections above via disjoint PSUM banks: 2+4+2=8) -------
                with (
                    tc.tile_pool(name=f"pss_{rep}", bufs=2, space="PSUM") as pss,
                    tc.tile_pool(name=f"psav_{rep}", bufs=2, space="PSUM") as psav,
                ):
                    for pair in range(2):
                        if pair == 1:
                            for ib2 in range(NNB):
                                k_group(1, ib2)
                            for ib2 in range(NNB):
                                q_group(1, ib2)
                        for ib in range(NNB):
                            nq = slice(ib * NB, (ib + 1) * NB)
                            avp = [psav.tile([DH + 1, NB], F32, tag="av",
                                             name=f"avp{rep}_{pair}_{ib}_{h}")
                                   for h in range(2)]
                            for mi in range(NMI):
                                if pair == 0 and ib == 0:
                                    v_group(mi)
                                ms = slice(mi * P, (mi + 1) * P)
                                s = pss.tile([P, 2 * NB], F32, tag="s",
                                             name=f"s{rep}_{pair}_{ib}_{mi}")
                                es = espool.tile([P, 2 * NB], F32R, tag="es")
                                for half in range(2):
                                    d = slice(half * DH, (half + 1) * DH)
                                    nc.tensor.matmul(
                                        s[:, half * NB:(half + 1) * NB],
                                        KT[pair][d, ms], QT[pair][d, nq],
                                        start=True, stop=True)
                                nc.scalar.activation(es[:], s[:], EXP)
                                for half in range(2):
                                    nc.tensor.matmul(
                                        avp[half][:], Vn[:, mi, 2 * pair + half, :],
                                        es[:, half * NB:(half + 1) * NB],
                                        start=(mi == 0), stop=(mi == NMI - 1))
                            # drain AV psum quickly to SBUF, then normalize
                            # (PE K=1 matmul broadcasts 1/rowsum to 64 rows)
                            avu = []
                            for half in range(2):
                                u = roll.tile([DH + 1, NB], F32, tag="avu")
                                nc.vector.tensor_copy(u[:], avp[half][:])
                                avu.append(u)
                            for half in range(2):
                                rr = roll2.tile([1, NB], F32R, tag="rr")
                                with nc.allow_low_precision(
                                        reason="f32r tag for PE broadcast; "
                                               "values are fp32"):
                                    nc.vector.reciprocal(rr[:], avu[half][DH:DH + 1, :])
                                rbp = psav.tile([DH, NB], F32, tag="av",
                                                name=f"rbp{rep}_{pair}_{ib}_{half}")
                                nc.tensor.matmul(rbp[:], onesrow[:], rr[:],
                                                 start=True, stop=True)
                                nc.vector.tensor_mul(
                                    AVn[pair][half * DH:(half + 1) * DH, nq],
                                    rbp[:], avu[half][0:DH, :])

                            # ---- phase 4: out-proj for this nq block,
                            # overlapped with later attention blocks (reuses
                            # the now mostly idle ps1 slots) ----------------
                            if pair == 1:
                                for nqi in range(ib * 4, ib * 4 + 4):
                                    for co in range(2):
                                        po = ps1.tile([P, 512], F32, tag="p1",
                                                      name=f"po{rep}_{nqi}_{co}")
                                        for pr in range(2):
                                            nc.tensor.matmul(
                                                po[:],
                                                AVn[pr][:, nqi * P:(nqi + 1) * P],
                                                wos[:, pr, co * 512:(co + 1) * 512],
                                                start=(pr == 0), stop=(pr == 1))
                                        ot = roll2.tile([P, 512], F32, tag="ot")
                                        nc.vector.tensor_copy(ot[:], po[:])
                                        nc.sync.dma_start(
                                            outp[nqi * P:(nqi + 1) * P,
                                                 co * 512:(co + 1) * 512],
                                            ot[:])
                                        if tiny_out and nqi == NMI - 1 and co == 1:
                                            nc.sync.dma_start(outp_t[:], ot[:])

    nc.compile()
    return nc


def kernel(x, attention_mask, Wqkv, bqkv, We, be, Wo, bo):
    x = np.asarray(x, dtype=np.float32)
    Wqkv = np.asarray(Wqkv, dtype=np.float32)
    We = np.asarray(We, dtype=np.float32)
    Wo = np.asarray(Wo, dtype=np.float32)

    if "nc" not in _CACHE:
        _CACHE["nc"] = _build()
    nc = _CACHE["nc"]

    in_maps = []
    for c in range(8):
        b, g = divmod(c, 4)
        cols = slice(g * HPC * DH, (g + 1) * HPC * DH)
        in_maps.append({
            "xT": np.ascontiguousarray(x[b].T),
            "wq": np.ascontiguousarray(Wqkv[:, 0 * C:1 * C][:, cols]),
            "wk": np.ascontiguousarray(Wqkv[:, 1 * C:2 * C][:, cols]),
            "wv": np.ascontiguousarray(Wqkv[:, 2 * C:3 * C][:, cols]),
            "we": np.ascontiguousarray(We[:, g * HPC:(g + 1) * HPC]),
            "wo": np.ascontiguousarray(Wo[cols, :]),
            "ones64": np.ones((P, NMI * HPC), dtype=np.float32),
        })

    trace = bool(int(os.environ.get("KERNEL_TRACE", "0")))
    res = run_bass_kernel_spmd(nc, in_maps, core_ids=list(range(8)), trace=trace)
    _CACHE["last_result"] = res

    parts = [res.results[c]["outp"] for c in range(8)]
    out = np.stack([parts[0] + parts[1] + parts[2] + parts[3],
                    parts[4] + parts[5] + parts[6] + parts[7]])
    out += np.asarray(bo, dtype=np.float32)
    return out.astype(np.float32)



# revision 9
# speedup vs baseline: 1.2524x; 1.2524x over previous
"""EntropyGuidedAttention Trainium2 kernel (v2).

B=2, N=2048, C=1024, H=16, Dh=64 on 8 NeuronCores:
data-parallel over batch (cores 0-3 -> batch 0, 4-7 -> batch 1), tensor-parallel
over heads within a batch group (4 heads per core). Each core computes its
heads' attention and a row-split partial of the output projection; the host
sums the 4 partials per batch.

v2 layout/schedule (vs v1):
- bf16 SBUF datapath everywhere (x, weights, Q/K/V, exp(S), AV, out-proj
  inputs); PSUM accumulation stays fp32. Halves input DMA and DVE traffic.
- softmax normalize runs entirely off the PE: ones-column rowsums ->
  DVE reciprocal [1,512] -> GPSIMD partition_broadcast -> DVE multiply.
  (v1 used a K=1 PE matmul fed by the reciprocal, which head-of-line
  blocked the PE queue for ~3us per block and re-throttled the HAM clock.)
- the sigmoid gate row-broadcast stays on the DMA engines (DRAM staging
  round-trip; GPSIMD partition_broadcast only accepts 0/32/64/96-aligned
  source partitions, and the gate rows live on partitions 0-3).
- minimal prelude (gate/K/Q for the first block only) + a filler-task queue:
  remaining projections, V, and the output projection are interleaved one
  small burst per attention step, so the scalar engine (exp, the real
  bottleneck at ~147us) never starves and the PE never idles long enough
  to lose the HAM 2.4GHz clock.
"""
import os
import sys

sys.path.insert(0, "/opt/trn_rl_repo")

import numpy as np
import ml_dtypes

import concourse.bass as bass
import concourse.mybir as mybir
import concourse.tile as tile
from concourse import bacc, library_config
from concourse.bass_utils import run_bass_kernel_spmd

F32 = mybir.dt.float32
BF16 = mybir.dt.bfloat16
EXP = mybir.ActivationFunctionType.Exp
SIGMOID = mybir.ActivationFunctionType.Sigmoid

B, N, C, H = 2, 2048, 1024, 16
DH = C // H          # 64
HPC = 4              # heads per core
PW = 2 * DH          # head-pair width = 128
P = 128
NCI = C // P         # 8 contraction chunks
NNB = 4              # nq blocks
NB = 512             # nq block size
NMI = N // P         # 16 m-chunks
SCALE = 1.0 / 8.0    # 1/sqrt(DH)

_CACHE = {}


def _bcast_rows(nc, dst, row, nrows):
    """DMA-broadcast a [1, W] DRAM row across `nrows` SBUF partitions."""
    src = bass.AP(tensor=row.tensor, offset=row.offset,
                  ap=[[0, nrows]] + list(row.ap[1:]))
    nc.sync.dma_start(dst, src)


def _build(reps=1):
    nc = bacc.Bacc("TRN2", target_bir_lowering=False, debug=False, num_devices=8)

    xT = nc.dram_tensor("xT", [C, N], BF16, kind="ExternalInput")
    wq = nc.dram_tensor("wq", [C, HPC * DH], BF16, kind="ExternalInput")
    wk = nc.dram_tensor("wk", [C, HPC * DH], BF16, kind="ExternalInput")
    wv = nc.dram_tensor("wv", [C, HPC * DH], BF16, kind="ExternalInput")
    we = nc.dram_tensor("we", [C, HPC], BF16, kind="ExternalInput")
    wo = nc.dram_tensor("wo", [HPC * DH, C], BF16, kind="ExternalInput")
    ones64 = nc.dram_tensor("ones64", [P, NMI * HPC], BF16, kind="ExternalInput")
    outp = nc.dram_tensor("outp", [N, C], F32, kind="ExternalOutput")

    with tile.TileContext(nc) as tc, (
        tc.tile_pool(name="big", bufs=1)) as big, (
        tc.tile_pool(name="roll", bufs=3)) as roll, (
        tc.tile_pool(name="roll2", bufs=3)) as roll2, (
        tc.tile_pool(name="espool", bufs=4)) as espool, (
        tc.tile_pool(name="dram", bufs=1, space="DRAM")) as dram:
        nc.gpsimd.load_library(library_config.attn)
        for rep in range(reps):
            # ---- resident SBUF inputs; DMA priority order -------------------
            wes = big.tile([P, NCI, HPC], BF16, tag="wes", name=f"wes{rep}")
            nc.sync.dma_start(wes[:], we.rearrange("(o p) f -> p o f", p=P))
            wks = big.tile([P, NCI, HPC * DH], BF16, tag="wks", name=f"wks{rep}")
            nc.sync.dma_start(wks[:], wk.rearrange("(o p) f -> p o f", p=P))
            wqs = big.tile([P, NCI, HPC * DH], BF16, tag="wqs", name=f"wqs{rep}")
            nc.sync.dma_start(wqs[:], wq.rearrange("(o p) f -> p o f", p=P))
            xs = big.tile([P, NCI, N], BF16, tag="xs", name=f"xs{rep}")
            xTv = xT.rearrange("(o p) n -> p o n", p=P)
            for ci in range(NCI):
                nc.sync.dma_start(xs[:, ci, :], xTv[:, ci, :])
            wvs = big.tile([P, NCI, HPC * DH], BF16, tag="wvs", name=f"wvs{rep}")
            nc.sync.dma_start(wvs[:], wv.rearrange("(o p) f -> p o f", p=P))
            wos = big.tile([P, 2, C], BF16, tag="wos", name=f"wos{rep}")
            nc.sync.dma_start(wos[:], wo.rearrange("(o p) f -> p o f", p=P))

            KT = [big.tile([P, N], BF16, tag=f"kt{p}", name=f"kt{p}_{rep}")
                  for p in range(2)]
            QT = [big.tile([P, N], BF16, tag=f"qt{p}", name=f"qt{p}_{rep}")
                  for p in range(2)]
            Vn = big.tile([P, NMI, HPC, DH + 1], BF16, tag="vn", name=f"vn{rep}")
            E4 = big.tile([HPC, N], F32, tag="e4", name=f"e4{rep}")
            AVn = [big.tile([P, N], BF16, tag=f"avn{p}", name=f"avn{p}_{rep}")
                   for p in range(2)]
            estg = dram.tile([HPC, N], F32, tag="estg", name=f"estg{rep}")

            nc.sync.dma_start(
                Vn[:, :, :, DH:DH + 1],
                ones64[:].rearrange("p (m h) -> p m h", h=HPC)[:, :, :, None])

            with (
                tc.tile_pool(name=f"ps1_{rep}", bufs=2, space="PSUM") as ps1,
                tc.tile_pool(name=f"pss_{rep}", bufs=2, space="PSUM") as pss,
                tc.tile_pool(name=f"psav_{rep}", bufs=2, space="PSUM") as psav,
            ):
                # ---- building blocks ------------------------------------
                def gate_group(ib):
                    nq = slice(ib * NB, (ib + 1) * NB)
                    pe = ps1.tile([HPC, NB], F32, tag="p1", name=f"pe{rep}_{ib}")
                    for ci in range(NCI):
                        nc.tensor.matmul(pe[:], wes[:, ci, :], xs[:, ci, nq],
                                         start=(ci == 0), stop=(ci == NCI - 1))
                    nc.scalar.activation(E4[:, nq], pe[:], SIGMOID)
                    nc.vector.tensor_scalar_mul(E4[:, nq], E4[:, nq], SCALE)
                    nc.sync.dma_start(estg[:, nq], E4[:, nq])

                def k_group(pair, ib):
                    nq = slice(ib * NB, (ib + 1) * NB)
                    pk = ps1.tile([P, NB], F32, tag="p1", name=f"pk{rep}_{pair}_{ib}")
                    for ci in range(NCI):
                        nc.tensor.matmul(
                            pk[:], wks[:, ci, pair * PW:(pair + 1) * PW],
                            xs[:, ci, nq],
                            start=(ci == 0), stop=(ci == NCI - 1))
                    nc.vector.tensor_copy(KT[pair][:, nq], pk[:])

                def q_group(pair, ib):
                    nq = slice(ib * NB, (ib + 1) * NB)
                    pq = ps1.tile([P, NB], F32, tag="p1", name=f"pq{rep}_{pair}_{ib}")
                    for ci in range(NCI):
                        nc.tensor.matmul(
                            pq[:], wqs[:, ci, pair * PW:(pair + 1) * PW],
                            xs[:, ci, nq],
                            start=(ci == 0), stop=(ci == NCI - 1))
                    g = roll2.tile([P, NB], F32, tag="g")
                    for half in range(2):
                        _bcast_rows(nc, g[half * DH:(half + 1) * DH, :],
                                    estg[2 * pair + half:2 * pair + half + 1, nq],
                                    DH)
                    nc.vector.tensor_mul(QT[pair][:, nq], pq[:], g[:])

                def v_group(mi):
                    pv = ps1.tile([P, HPC * DH], F32, tag="p1", name=f"pv{rep}_{mi}")
                    for ci in range(NCI):
                        nc.tensor.matmul(pv[:], xs[:, ci, mi * P:(mi + 1) * P],
                                         wvs[:, ci, :],
                                         start=(ci == 0), stop=(ci == NCI - 1))
                    nc.vector.tensor_copy(Vn[:, mi, :, 0:DH],
                                          pv[:].rearrange("p (h d) -> p h d", h=HPC))

                def po_group(nqi, co):
                    po = ps1.tile([P, NB], F32, tag="p1", name=f"po{rep}_{nqi}_{co}")
                    for pr in range(2):
                        nc.tensor.matmul(
                            po[:], AVn[pr][:, nqi * P:(nqi + 1) * P],
                            wos[:, pr, co * NB:(co + 1) * NB],
                            start=(pr == 0), stop=(pr == 1))
                    ot = roll2.tile([P, NB], F32, tag="ot")
                    nc.vector.tensor_copy(ot[:], po[:])
                    nc.sync.dma_start(
                        outp[nqi * P:(nqi + 1) * P, co * NB:(co + 1) * NB], ot[:])

                def normalize(pair, ib, avp):
                    # rowsums (avp row DH) -> 1/r -> broadcast to 64 rows ->
                    # scale; PE-free (DVE + GPSIMD only).
                    nq = slice(ib * NB, (ib + 1) * NB)
                    avu = []
                    for half in range(2):
                        u = roll.tile([DH + 1, NB], F32, tag="avu")
                        nc.vector.tensor_copy(u[:], avp[half][:])
                        avu.append(u)
                    for half in range(2):
                        rr = roll2.tile([1, NB], F32, tag="rr")
                        nc.vector.reciprocal(rr[:], avu[half][DH:DH + 1, :])
                        gr = roll.tile([DH, NB], F32, tag="gr")
                        nc.gpsimd.partition_broadcast(gr[:], rr[:], channels=DH)
                        nc.vector.tensor_mul(
                            AVn[pair][half * DH:(half + 1) * DH, nq],
                            avu[half][0:DH, :], gr[:])

                # ---- filler task queue ----------------------------------
                # Emission order IS the engine queue order; a read must be
                # emitted after the write it consumes, so each block's
                # prerequisites are force-flushed before the block starts.
                fillers = []
                pumped = [0]

                def pump(k):
                    for _ in range(k):
                        if not fillers:
                            return
                        fn, args = fillers.pop(0)
                        fn(*args)
                        pumped[0] += 1

                def pump_until(k):
                    while pumped[0] < k and fillers:
                        pump(1)

                # ---- prelude: just enough for pair0/ib0 -----------------
                gate_group(0)
                for ib in range(NNB):
                    k_group(0, ib)
                q_group(0, 0)
                for mi in range(8):
                    v_group(mi)

                # ordered: 2 tasks per pair0 block boundary, then pair-1 K/Q
                for ib in range(1, NNB):
                    fillers.append((gate_group, (ib,)))
                    fillers.append((q_group, (0, ib)))
                for ib in range(NNB):
                    fillers.append((k_group, (1, ib)))
                for ib in range(NNB):
                    fillers.append((q_group, (1, ib)))

                # pump slots per (pair, ib): spaced mi indices
                slots = {
                    (0, 0): [6, 15],
                    (0, 1): [1, 4, 7, 10, 13],
                    (0, 2): [1, 4, 7, 10, 13],
                    (0, 3): [1, 4, 7, 10],
                    (1, 0): [1, 3, 5, 7, 9, 11, 13, 15],
                    (1, 1): [1, 3, 5, 7, 9, 11, 13, 15],
                    (1, 2): [1, 3, 5, 7, 9, 11, 13, 15],
                    (1, 3): [1, 3, 5, 7, 9, 11, 13, 15],
                }

                # ---- attention ------------------------------------------
                for pair in range(2):
                    for ib in range(NNB):
                        if pair == 0:
                            pump_until(2 * ib)   # gate(ib), q0(ib) emitted
                        elif ib == 0:
                            pump_until(14)       # all projections emitted
                        nq = slice(ib * NB, (ib + 1) * NB)
                        avp = [psav.tile([DH + 1, NB], F32, tag="av",
                                         name=f"avp{rep}_{pair}_{ib}_{h}")
                               for h in range(2)]
                        for mi in range(NMI):
                            if pair == 0 and ib == 0 and mi >= 8:
                                v_group(mi)
                            ms = slice(mi * P, (mi + 1) * P)
                            s = pss.tile([P, 2 * NB], F32, tag="s",
                                         name=f"s{rep}_{pair}_{ib}_{mi}")
                            es = espool.tile([P, 2 * NB], BF16, tag="es")
                            for half in range(2):
                                d = slice(half * DH, (half + 1) * DH)
                                nc.tensor.matmul(
                                    s[:, half * NB:(half + 1) * NB],
                                    KT[pair][d, ms], QT[pair][d, nq],
                                    start=True, stop=True)
                            nc.scalar.activation(es[:], s[:], EXP)
                            for half in range(2):
                                nc.tensor.matmul(
                                    avp[half][:], Vn[:, mi, 2 * pair + half, :],
                                    es[:, half * NB:(half + 1) * NB],
                                    start=(mi == 0), stop=(mi == NMI - 1))
                            if mi in slots[(pair, ib)]:
                                pump(1)
                        normalize(pair, ib, avp)
                        if pair == 1:
                            for nqi in range(ib * 4, ib * 4 + 4):
                                for co in range(2):
                                    fillers.append((po_group, (nqi, co)))
                pump(len(fillers))

    nc.compile()
    return nc


def _bf16(a):
    return np.ascontiguousarray(a).astype(ml_dtypes.bfloat16)


def make_in_maps(x, Wqkv, We, Wo):
    in_maps = []
    for c in range(8):
        b, g = divmod(c, 4)
        cols = slice(g * HPC * DH, (g + 1) * HPC * DH)
        in_maps.append({
            "xT": _bf16(x[b].T),
            "wq": _bf16(Wqkv[:, 0 * C:1 * C][:, cols]),
            "wk": _bf16(Wqkv[:, 1 * C:2 * C][:, cols]),
            "wv": _bf16(Wqkv[:, 2 * C:3 * C][:, cols]),
            "we": _bf16(We[:, g * HPC:(g + 1) * HPC]),
            "wo": _bf16(Wo[cols, :]),
            "ones64": np.ones((P, NMI * HPC), dtype=ml_dtypes.bfloat16),
        })
    return in_maps


def kernel(x, attention_mask, Wqkv, bqkv, We, be, Wo, bo):
    x = np.asarray(x, dtype=np.float32)
    Wqkv = np.asarray(Wqkv, dtype=np.float32)
    We = np.asarray(We, dtype=np.float32)
    Wo = np.asarray(Wo, dtype=np.float32)

    if "nc" not in _CACHE:
        _CACHE["nc"] = _build()
    nc = _CACHE["nc"]

    in_maps = make_in_maps(x, Wqkv, We, Wo)

    trace = bool(int(os.environ.get("KERNEL_TRACE", "0")))
    res = run_bass_kernel_spmd(nc, in_maps, core_ids=list(range(8)), trace=trace)
    _CACHE["last_result"] = res

    parts = [res.results[c]["outp"] for c in range(8)]
    out = np.stack([parts[0] + parts[1] + parts[2] + parts[3],
                    parts[4] + parts[5] + parts[6] + parts[7]])
    out += np.asarray(bo, dtype=np.float32)
    return out.astype(np.float32)


# revision 15
# speedup vs baseline: 1.2589x; 1.0051x over previous
"""EntropyGuidedAttention Trainium2 kernel (v2).

B=2, N=2048, C=1024, H=16, Dh=64 on 8 NeuronCores:
data-parallel over batch (cores 0-3 -> batch 0, 4-7 -> batch 1), tensor-parallel
over heads within a batch group (4 heads per core). Each core computes its
heads' attention and a row-split partial of the output projection; the host
sums the 4 partials per batch.

v2 layout/schedule (vs v1):
- bf16 SBUF datapath everywhere (x, weights, Q/K/V, exp(S), AV, out-proj
  inputs); PSUM accumulation stays fp32. Halves input DMA and DVE traffic.
- softmax normalize runs entirely off the PE: ones-column rowsums ->
  DVE reciprocal [1,512] -> GPSIMD partition_broadcast -> DVE multiply.
  (v1 used a K=1 PE matmul fed by the reciprocal, which head-of-line
  blocked the PE queue for ~3us per block and re-throttled the HAM clock.)
- the sigmoid gate row-broadcast stays on the DMA engines (DRAM staging
  round-trip; GPSIMD partition_broadcast only accepts 0/32/64/96-aligned
  source partitions, and the gate rows live on partitions 0-3).
- minimal prelude (gate/K/Q for the first block only) + a filler-task queue:
  remaining projections, V, and the output projection are interleaved one
  small burst per attention step, so the scalar engine (exp, the real
  bottleneck at ~147us) never starves and the PE never idles long enough
  to lose the HAM 2.4GHz clock.
"""
import os
import sys

sys.path.insert(0, "/opt/trn_rl_repo")

import numpy as np
import ml_dtypes

import concourse.bass as bass
import concourse.mybir as mybir
import concourse.tile as tile
from concourse import bacc, library_config
from concourse.bass_utils import run_bass_kernel_spmd

F32 = mybir.dt.float32
BF16 = mybir.dt.bfloat16
EXP = mybir.ActivationFunctionType.Exp
SIGMOID = mybir.ActivationFunctionType.Sigmoid

B, N, C, H = 2, 2048, 1024, 16
DH = C // H          # 64
HPC = 4              # heads per core
PW = 2 * DH          # head-pair width = 128
P = 128
NCI = C // P         # 8 contraction chunks
NNB = 4              # nq blocks
NB = 512             # nq block size
NMI = N // P         # 16 m-chunks
SCALE = 1.0 / 8.0    # 1/sqrt(DH)

_CACHE = {}


def _bcast_rows(nc, dst, row, nrows):
    """DMA-broadcast a [1, W] DRAM row across `nrows` SBUF partitions."""
    src = bass.AP(tensor=row.tensor, offset=row.offset,
                  ap=[[0, nrows]] + list(row.ap[1:]))
    nc.sync.dma_start(dst, src)


def _build(reps=1):
    nc = bacc.Bacc("TRN2", target_bir_lowering=False, debug=False, num_devices=8)

    xT = nc.dram_tensor("xT", [C, N], BF16, kind="ExternalInput")
    wq = nc.dram_tensor("wq", [C, HPC * DH], BF16, kind="ExternalInput")
    wk = nc.dram_tensor("wk", [C, HPC * DH], BF16, kind="ExternalInput")
    wv = nc.dram_tensor("wv", [C, HPC * DH], BF16, kind="ExternalInput")
    we = nc.dram_tensor("we", [C, HPC], BF16, kind="ExternalInput")
    wo = nc.dram_tensor("wo", [HPC * DH, C], BF16, kind="ExternalInput")
    ones64 = nc.dram_tensor("ones64", [P, NMI * HPC], BF16, kind="ExternalInput")
    outp = nc.dram_tensor("outp", [N, C], BF16, kind="ExternalOutput")

    with tile.TileContext(nc) as tc, (
        tc.tile_pool(name="big", bufs=1)) as big, (
        tc.tile_pool(name="roll", bufs=3)) as roll, (
        tc.tile_pool(name="roll2", bufs=3)) as roll2, (
        tc.tile_pool(name="espool", bufs=4)) as espool, (
        tc.tile_pool(name="dram", bufs=1, space="DRAM")) as dram:
        nc.gpsimd.load_library(library_config.attn)
        for rep in range(reps):
            # ---- resident SBUF inputs; DMA priority order -------------------
            wes = big.tile([P, NCI, HPC], BF16, tag="wes", name=f"wes{rep}")
            nc.sync.dma_start(wes[:], we.rearrange("(o p) f -> p o f", p=P))
            wks = big.tile([P, NCI, HPC * DH], BF16, tag="wks", name=f"wks{rep}")
            nc.sync.dma_start(wks[:], wk.rearrange("(o p) f -> p o f", p=P))
            wqs = big.tile([P, NCI, HPC * DH], BF16, tag="wqs", name=f"wqs{rep}")
            nc.sync.dma_start(wqs[:], wq.rearrange("(o p) f -> p o f", p=P))
            xs = big.tile([P, NCI, N], BF16, tag="xs", name=f"xs{rep}")
            xTv = xT.rearrange("(o p) n -> p o n", p=P)
            for ci in range(NCI):
                nc.sync.dma_start(xs[:, ci, :], xTv[:, ci, :])
            wvs = big.tile([P, NCI, HPC * DH], BF16, tag="wvs", name=f"wvs{rep}")
            nc.sync.dma_start(wvs[:], wv.rearrange("(o p) f -> p o f", p=P))
            wos = big.tile([P, 2, C], BF16, tag="wos", name=f"wos{rep}")
            nc.sync.dma_start(wos[:], wo.rearrange("(o p) f -> p o f", p=P))

            KT = [big.tile([P, N], BF16, tag=f"kt{p}", name=f"kt{p}_{rep}")
                  for p in range(2)]
            QT = [big.tile([P, N], BF16, tag=f"qt{p}", name=f"qt{p}_{rep}")
                  for p in range(2)]
            Vn = big.tile([P, NMI, HPC, DH + 1], BF16, tag="vn", name=f"vn{rep}")
            E4 = big.tile([HPC, N], F32, tag="e4", name=f"e4{rep}")
            AVn = [big.tile([P, N], BF16, tag=f"avn{p}", name=f"avn{p}_{rep}")
                   for p in range(2)]
            estg = dram.tile([HPC, N], F32, tag="estg", name=f"estg{rep}")

            nc.sync.dma_start(
                Vn[:, :, :, DH:DH + 1],
                ones64[:].rearrange("p (m h) -> p m h", h=HPC)[:, :, :, None])

            with (
                tc.tile_pool(name=f"ps1_{rep}", bufs=2, space="PSUM") as ps1,
                tc.tile_pool(name=f"pss_{rep}", bufs=2, space="PSUM") as pss,
                tc.tile_pool(name=f"psav_{rep}", bufs=2, space="PSUM") as psav,
            ):
                # ---- building blocks ------------------------------------
                def gate_group(ib):
                    nq = slice(ib * NB, (ib + 1) * NB)
                    pe = ps1.tile([HPC, NB], F32, tag="p1", name=f"pe{rep}_{ib}")
                    for ci in range(NCI):
                        nc.tensor.matmul(pe[:], wes[:, ci, :], xs[:, ci, nq],
                                         start=(ci == 0), stop=(ci == NCI - 1))
                    nc.scalar.activation(E4[:, nq], pe[:], SIGMOID)
                    nc.vector.tensor_scalar_mul(E4[:, nq], E4[:, nq], SCALE)
                    nc.sync.dma_start(estg[:, nq], E4[:, nq])

                def k_group(pair, ib):
                    nq = slice(ib * NB, (ib + 1) * NB)
                    pk = ps1.tile([P, NB], F32, tag="p1", name=f"pk{rep}_{pair}_{ib}")
                    for ci in range(NCI):
                        nc.tensor.matmul(
                            pk[:], wks[:, ci, pair * PW:(pair + 1) * PW],
                            xs[:, ci, nq],
                            start=(ci == 0), stop=(ci == NCI - 1))
                    nc.vector.tensor_copy(KT[pair][:, nq], pk[:])

                def q_group(pair, ib):
                    nq = slice(ib * NB, (ib + 1) * NB)
                    pq = ps1.tile([P, NB], F32, tag="p1", name=f"pq{rep}_{pair}_{ib}")
                    for ci in range(NCI):
                        nc.tensor.matmul(
                            pq[:], wqs[:, ci, pair * PW:(pair + 1) * PW],
                            xs[:, ci, nq],
                            start=(ci == 0), stop=(ci == NCI - 1))
                    g = roll2.tile([P, NB], F32, tag="g")
                    for half in range(2):
                        _bcast_rows(nc, g[half * DH:(half + 1) * DH, :],
                                    estg[2 * pair + half:2 * pair + half + 1, nq],
                                    DH)
                    nc.vector.tensor_mul(QT[pair][:, nq], pq[:], g[:])

                def v_group(pair, mi):
                    # one head-pair's V for m-chunk mi (pair p covers heads
                    # 2p..2p+1); split so each pair's ib0 loads only its V.
                    pv = ps1.tile([P, PW], F32, tag="p1", name=f"pv{rep}_{pair}_{mi}")
                    for ci in range(NCI):
                        nc.tensor.matmul(pv[:], xs[:, ci, mi * P:(mi + 1) * P],
                                         wvs[:, ci, pair * PW:(pair + 1) * PW],
                                         start=(ci == 0), stop=(ci == NCI - 1))
                    nc.vector.tensor_copy(
                        Vn[:, mi, 2 * pair:2 * pair + 2, 0:DH],
                        pv[:].rearrange("p (h d) -> p h d", h=2))

                def po_group(nqi, co):
                    po = ps1.tile([P, NB], F32, tag="p1", name=f"po{rep}_{nqi}_{co}")
                    for pr in range(2):
                        nc.tensor.matmul(
                            po[:], AVn[pr][:, nqi * P:(nqi + 1) * P],
                            wos[:, pr, co * NB:(co + 1) * NB],
                            start=(pr == 0), stop=(pr == 1))
                    ot = roll2.tile([P, NB], BF16, tag="ot")
                    with nc.allow_low_precision(reason="bf16 output partials"):
                        nc.vector.tensor_copy(ot[:], po[:])
                    nc.sync.dma_start(
                        outp[nqi * P:(nqi + 1) * P, co * NB:(co + 1) * NB], ot[:])

                def normalize(pair, ib, avp):
                    # rowsums (avp row DH) -> 1/r (bf16) -> GPSIMD broadcast
                    # to 64 rows -> scale; PE-free (DVE + GPSIMD only).
                    nq = slice(ib * NB, (ib + 1) * NB)
                    avu, rrs = [], []
                    for half in range(2):
                        u = roll.tile([DH + 1, NB], F32, tag="avu")
                        nc.vector.tensor_copy(u[:], avp[half][:])
                        avu.append(u)
                    for half in range(2):
                        rr = roll2.tile([1, NB], BF16, tag="rr")
                        with nc.allow_low_precision(reason="softmax denom recip"):
                            nc.vector.reciprocal(rr[:], avu[half][DH:DH + 1, :])
                        rrs.append(rr)
                    grs = []
                    for half in range(2):
                        gr = roll.tile([DH, NB], BF16, tag="gr")
                        nc.gpsimd.partition_broadcast(gr[:], rrs[half][:], channels=DH)
                        grs.append(gr)
                    for half in range(2):
                        with nc.allow_low_precision(reason="normalize scale"):
                            nc.vector.tensor_mul(
                                AVn[pair][half * DH:(half + 1) * DH, nq],
                                avu[half][0:DH, :], grs[half][:])

                # ---- filler task queue ----------------------------------
                # Emission order IS the engine queue order; a read must be
                # emitted after the write it consumes, so each block's
                # prerequisites are force-flushed before the block starts.
                fillers = []
                pumped = [0]

                def pump(k):
                    for _ in range(k):
                        if not fillers:
                            return
                        fn, args = fillers.pop(0)
                        fn(*args)
                        pumped[0] += 1

                def pump_until(k):
                    while pumped[0] < k and fillers:
                        pump(1)

                # ---- prelude: just enough for pair0/ib0 -----------------
                gate_group(0)
                for ib in range(NNB):
                    k_group(0, ib)
                q_group(0, 0)

                # ordered: 2 tasks per pair0 block boundary, then pair-1 K/Q
                for ib in range(1, NNB):
                    fillers.append((gate_group, (ib,)))
                    fillers.append((q_group, (0, ib)))
                for ib in range(NNB):
                    fillers.append((k_group, (1, ib)))
                for ib in range(NNB):
                    fillers.append((q_group, (1, ib)))

                # pump slots per (pair, ib): spaced mi indices. Each pair's
                # ib0 carries its V-projection inline; pair-1 po slots sit in
                # the back half so the normalize chain (recip+broadcast) of
                # the previous block has landed.
                slots = {
                    (0, 0): [15],
                    (0, 1): [1, 4, 7, 10, 13],
                    (0, 2): [1, 4, 7, 10, 13],
                    (0, 3): [1, 4, 7, 10],
                    (1, 0): [],
                    (1, 1): [8, 9, 10, 11, 12, 13, 14, 15],
                    (1, 2): [8, 9, 10, 11, 12, 13, 14, 15],
                    (1, 3): [8, 9, 10, 11, 12, 13, 14, 15],
                }

                # ---- attention ------------------------------------------
                for pair in range(2):
                    for ib in range(NNB):
                        if pair == 0:
                            pump_until(2 * ib)   # gate(ib), q0(ib) emitted
                        elif ib == 0:
                            pump_until(14)       # all projections emitted
                        nq = slice(ib * NB, (ib + 1) * NB)
                        avp = [psav.tile([DH + 1, NB], F32, tag="av",
                                         name=f"avp{rep}_{pair}_{ib}_{h}")
                               for h in range(2)]
                        for mi in range(NMI):
                            if ib == 0:
                                v_group(pair, mi)
                            ms = slice(mi * P, (mi + 1) * P)
                            s = pss.tile([P, 2 * NB], F32, tag="s",
                                         name=f"s{rep}_{pair}_{ib}_{mi}")
                            es = espool.tile([P, 2 * NB], BF16, tag="es")
                            for half in range(2):
                                d = slice(half * DH, (half + 1) * DH)
                                nc.tensor.matmul(
                                    s[:, half * NB:(half + 1) * NB],
                                    KT[pair][d, ms], QT[pair][d, nq],
                                    start=True, stop=True)
                            nc.scalar.activation(es[:], s[:], EXP)
                            for half in range(2):
                                nc.tensor.matmul(
                                    avp[half][:], Vn[:, mi, 2 * pair + half, :],
                                    es[:, half * NB:(half + 1) * NB],
                                    start=(mi == 0), stop=(mi == NMI - 1))
                            if mi in slots[(pair, ib)]:
                                pump(1)
                        normalize(pair, ib, avp)
                        if pair == 1:
                            for nqi in range(ib * 4, ib * 4 + 4):
                                for co in range(2):
                                    fillers.append((po_group, (nqi, co)))
                pump(len(fillers))

    nc.compile()
    return nc


def _bf16(a):
    return np.ascontiguousarray(a).astype(ml_dtypes.bfloat16)


def make_in_maps(x, Wqkv, We, Wo):
    in_maps = []
    for c in range(8):
        b, g = divmod(c, 4)
        cols = slice(g * HPC * DH, (g + 1) * HPC * DH)
        in_maps.append({
            "xT": _bf16(x[b].T),
            "wq": _bf16(Wqkv[:, 0 * C:1 * C][:, cols]),
            "wk": _bf16(Wqkv[:, 1 * C:2 * C][:, cols]),
            "wv": _bf16(Wqkv[:, 2 * C:3 * C][:, cols]),
            "we": _bf16(We[:, g * HPC:(g + 1) * HPC]),
            "wo": _bf16(Wo[cols, :]),
            "ones64": np.ones((P, NMI * HPC), dtype=ml_dtypes.bfloat16),
        })
    return in_maps


def kernel(x, attention_mask, Wqkv, bqkv, We, be, Wo, bo):
    x = np.asarray(x, dtype=np.float32)
    Wqkv = np.asarray(Wqkv, dtype=np.float32)
    We = np.asarray(We, dtype=np.float32)
    Wo = np.asarray(Wo, dtype=np.float32)

    if "nc" not in _CACHE:
        _CACHE["nc"] = _build()
    nc = _CACHE["nc"]

    in_maps = make_in_maps(x, Wqkv, We, Wo)

    trace = bool(int(os.environ.get("KERNEL_TRACE", "0")))
    res = run_bass_kernel_spmd(nc, in_maps, core_ids=list(range(8)), trace=trace)
    _CACHE["last_result"] = res

    parts = [np.asarray(res.results[c]["outp"]).astype(np.float32)
             for c in range(8)]
    out = np.stack([parts[0] + parts[1] + parts[2] + parts[3],
                    parts[4] + parts[5] + parts[6] + parts[7]])
    out += np.asarray(bo, dtype=np.float32)
    return out.astype(np.float32)


# revision 19
# speedup vs baseline: 1.4686x; 1.1666x over previous
"""EntropyGuidedAttention Trainium2 kernel (v2).

B=2, N=2048, C=1024, H=16, Dh=64 on 8 NeuronCores:
data-parallel over batch (cores 0-3 -> batch 0, 4-7 -> batch 1), tensor-parallel
over heads within a batch group (4 heads per core). Each core computes its
heads' attention and a row-split partial of the output projection; the host
sums the 4 partials per batch.

v2 layout/schedule (vs v1):
- bf16 SBUF datapath everywhere (x, weights, Q/K/V, exp(S), AV, out-proj
  inputs); PSUM accumulation stays fp32. Halves input DMA and DVE traffic.
- softmax normalize runs entirely off the PE: ones-column rowsums ->
  DVE reciprocal [1,512] -> GPSIMD partition_broadcast -> DVE multiply.
  (v1 used a K=1 PE matmul fed by the reciprocal, which head-of-line
  blocked the PE queue for ~3us per block and re-throttled the HAM clock.)
- the sigmoid gate row-broadcast stays on the DMA engines (DRAM staging
  round-trip; GPSIMD partition_broadcast only accepts 0/32/64/96-aligned
  source partitions, and the gate rows live on partitions 0-3).
- minimal prelude (gate/K/Q for the first block only) + a filler-task queue:
  remaining projections, V, and the output projection are interleaved one
  small burst per attention step, so the scalar engine (exp, the real
  bottleneck at ~147us) never starves and the PE never idles long enough
  to lose the HAM 2.4GHz clock.
"""
import os
import sys

sys.path.insert(0, "/opt/trn_rl_repo")

import numpy as np
import ml_dtypes

import concourse.bass as bass
import concourse.mybir as mybir
import concourse.tile as tile
from concourse import bacc, library_config
from concourse.bass_utils import run_bass_kernel_spmd

F32 = mybir.dt.float32
BF16 = mybir.dt.bfloat16
EXP = mybir.ActivationFunctionType.Exp
TANH = mybir.ActivationFunctionType.Tanh

B, N, C, H = 2, 2048, 1024, 16
DH = C // H          # 64
HPC = 4              # heads per core
PW = 2 * DH          # head-pair width = 128
P = 128
NCI = C // P         # 8 contraction chunks
NNB = 4              # nq blocks
NB = 512             # nq block size
NMI = N // P         # 16 m-chunks
SCALE = 1.0 / 8.0    # 1/sqrt(DH)

_CACHE = {}


def _bcast_rows(nc, dst, row, nrows):
    """DMA-broadcast a [1, W] DRAM row across `nrows` SBUF partitions."""
    src = bass.AP(tensor=row.tensor, offset=row.offset,
                  ap=[[0, nrows]] + list(row.ap[1:]))
    nc.sync.dma_start(dst, src)


def _build(reps=1):
    nc = bacc.Bacc("TRN2", target_bir_lowering=False, debug=False, num_devices=8)

    xT = nc.dram_tensor("xT", [C, N], BF16, kind="ExternalInput")
    wq = nc.dram_tensor("wq", [C, HPC * DH], BF16, kind="ExternalInput")
    wk = nc.dram_tensor("wk", [C, HPC * DH], BF16, kind="ExternalInput")
    wv = nc.dram_tensor("wv", [C, HPC * DH], BF16, kind="ExternalInput")
    we = nc.dram_tensor("we", [C, HPC], BF16, kind="ExternalInput")
    wo = nc.dram_tensor("wo", [HPC * DH, C], BF16, kind="ExternalInput")
    ones64 = nc.dram_tensor("ones64", [P, NMI * HPC], BF16, kind="ExternalInput")
    outp = nc.dram_tensor("outp", [N, C], BF16, kind="ExternalOutput")

    with tile.TileContext(nc) as tc, (
        tc.tile_pool(name="big", bufs=1)) as big, (
        tc.tile_pool(name="roll", bufs=3)) as roll, (
        tc.tile_pool(name="roll2", bufs=3)) as roll2, (
        tc.tile_pool(name="espool", bufs=4)) as espool, (
        tc.tile_pool(name="dram", bufs=1, space="DRAM")) as dram:
        nc.gpsimd.load_library(library_config.attn)
        for rep in range(reps):
            # ---- resident SBUF inputs; DMA priority order -------------------
            wes = big.tile([P, NCI, HPC], BF16, tag="wes", name=f"wes{rep}")
            nc.sync.dma_start(wes[:], we.rearrange("(o p) f -> p o f", p=P))
            wks = big.tile([P, NCI, HPC * DH], BF16, tag="wks", name=f"wks{rep}")
            nc.sync.dma_start(wks[:], wk.rearrange("(o p) f -> p o f", p=P))
            wqs = big.tile([P, NCI, HPC * DH], BF16, tag="wqs", name=f"wqs{rep}")
            nc.sync.dma_start(wqs[:], wq.rearrange("(o p) f -> p o f", p=P))
            xs = big.tile([P, NCI, N], BF16, tag="xs", name=f"xs{rep}")
            xTv = xT.rearrange("(o p) n -> p o n", p=P)
            for ci in range(NCI):
                nc.sync.dma_start(xs[:, ci, :], xTv[:, ci, :])
            wvs = big.tile([P, NCI, HPC * DH], BF16, tag="wvs", name=f"wvs{rep}")
            nc.sync.dma_start(wvs[:], wv.rearrange("(o p) f -> p o f", p=P))
            wos = big.tile([P, 2, C], BF16, tag="wos", name=f"wos{rep}")
            nc.sync.dma_start(wos[:], wo.rearrange("(o p) f -> p o f", p=P))

            KT = [big.tile([P, N], BF16, tag=f"kt{p}", name=f"kt{p}_{rep}")
                  for p in range(2)]
            QT = [big.tile([P, N], BF16, tag=f"qt{p}", name=f"qt{p}_{rep}")
                  for p in range(2)]
            Vn = big.tile([P, NMI, HPC, DH + 1], BF16, tag="vn", name=f"vn{rep}")
            E4 = big.tile([HPC, N], F32, tag="e4", name=f"e4{rep}")
            AVn = [big.tile([P, N], BF16, tag=f"avn{p}", name=f"avn{p}_{rep}")
                   for p in range(2)]
            estg = dram.tile([HPC, N], F32, tag="estg", name=f"estg{rep}")

            nc.sync.dma_start(
                Vn[:, :, :, DH:DH + 1],
                ones64[:].rearrange("p (m h) -> p m h", h=HPC)[:, :, :, None])

            with (
                tc.tile_pool(name=f"ps1_{rep}", bufs=2, space="PSUM") as ps1,
                tc.tile_pool(name=f"pss_{rep}", bufs=2, space="PSUM") as pss,
                tc.tile_pool(name=f"psav_{rep}", bufs=2, space="PSUM") as psav,
            ):
                # ---- building blocks ------------------------------------
                def gate_group(ib):
                    nq = slice(ib * NB, (ib + 1) * NB)
                    pe = ps1.tile([HPC, NB], F32, tag="p1", name=f"pe{rep}_{ib}")
                    for ci in range(NCI):
                        nc.tensor.matmul(pe[:], wes[:, ci, :], xs[:, ci, nq],
                                         start=(ci == 0), stop=(ci == NCI - 1))
                    # sigmoid(x)/8 == (0.5 + 0.5*tanh(x/2))/8; tanh shares the
                    # exp table set, so the gate never forces an ACT
                    # table-set reload mid-attention (sigmoid would).
                    nc.scalar.activation(E4[:, nq], pe[:], TANH, scale=0.5)
                    nc.vector.tensor_scalar_mul(E4[:, nq], E4[:, nq], SCALE / 2)
                    nc.vector.tensor_scalar_add(E4[:, nq], E4[:, nq], SCALE / 2)
                    nc.sync.dma_start(estg[:, nq], E4[:, nq])

                def k_group(pair, ib):
                    nq = slice(ib * NB, (ib + 1) * NB)
                    pk = ps1.tile([P, NB], F32, tag="p1", name=f"pk{rep}_{pair}_{ib}")
                    for ci in range(NCI):
                        nc.tensor.matmul(
                            pk[:], wks[:, ci, pair * PW:(pair + 1) * PW],
                            xs[:, ci, nq],
                            start=(ci == 0), stop=(ci == NCI - 1))
                    nc.vector.tensor_copy(KT[pair][:, nq], pk[:])

                def q_group(pair, ib):
                    nq = slice(ib * NB, (ib + 1) * NB)
                    pq = ps1.tile([P, NB], F32, tag="p1", name=f"pq{rep}_{pair}_{ib}")
                    for ci in range(NCI):
                        nc.tensor.matmul(
                            pq[:], wqs[:, ci, pair * PW:(pair + 1) * PW],
                            xs[:, ci, nq],
                            start=(ci == 0), stop=(ci == NCI - 1))
                    g = roll2.tile([P, NB], F32, tag="g")
                    for half in range(2):
                        _bcast_rows(nc, g[half * DH:(half + 1) * DH, :],
                                    estg[2 * pair + half:2 * pair + half + 1, nq],
                                    DH)
                    nc.vector.tensor_mul(QT[pair][:, nq], pq[:], g[:])

                def v_group(pair, mi):
                    # one head-pair's V for m-chunk mi (pair p covers heads
                    # 2p..2p+1); split so each pair's ib0 loads only its V.
                    pv = ps1.tile([P, PW], F32, tag="p1", name=f"pv{rep}_{pair}_{mi}")
                    for ci in range(NCI):
                        nc.tensor.matmul(pv[:], xs[:, ci, mi * P:(mi + 1) * P],
                                         wvs[:, ci, pair * PW:(pair + 1) * PW],
                                         start=(ci == 0), stop=(ci == NCI - 1))
                    nc.vector.tensor_copy(
                        Vn[:, mi, 2 * pair:2 * pair + 2, 0:DH],
                        pv[:].rearrange("p (h d) -> p h d", h=2))

                def po_group(nqi, co):
                    po = ps1.tile([P, NB], F32, tag="p1", name=f"po{rep}_{nqi}_{co}")
                    for pr in range(2):
                        nc.tensor.matmul(
                            po[:], AVn[pr][:, nqi * P:(nqi + 1) * P],
                            wos[:, pr, co * NB:(co + 1) * NB],
                            start=(pr == 0), stop=(pr == 1))
                    ot = roll2.tile([P, NB], BF16, tag="ot")
                    with nc.allow_low_precision(reason="bf16 output partials"):
                        nc.vector.tensor_copy(ot[:], po[:])
                    nc.sync.dma_start(
                        outp[nqi * P:(nqi + 1) * P, co * NB:(co + 1) * NB], ot[:])

                def normalize(pair, ib, avp):
                    # rowsums (avp row DH) -> 1/r (fast NR reciprocal) ->
                    # GPSIMD broadcast to 64 rows -> scale; PE-free.
                    nq = slice(ib * NB, (ib + 1) * NB)
                    avu = []
                    for half in range(2):
                        u = roll.tile([DH + 1, NB], F32, tag="avu")
                        nc.vector.tensor_copy(u[:], avp[half][:])
                        avu.append(u)
                    grs = []
                    for half in range(2):
                        rr = roll2.tile([1, NB], F32, tag="rr")
                        nc.vector.reciprocal_approx_fast(rr[:], avu[half][DH:DH + 1, :])
                        gr = roll.tile([DH, NB], F32, tag="gr")
                        nc.gpsimd.partition_broadcast(gr[:], rr[:], channels=DH)
                        grs.append(gr)
                    for half in range(2):
                        nc.vector.tensor_mul(
                            AVn[pair][half * DH:(half + 1) * DH, nq],
                            avu[half][0:DH, :], grs[half][:])

                # ---- filler task queue ----------------------------------
                # Emission order IS the engine queue order; a read must be
                # emitted after the write it consumes, so each block's
                # prerequisites are force-flushed before the block starts.
                fillers = []
                pumped = [0]

                def pump(k):
                    for _ in range(k):
                        if not fillers:
                            return
                        fn, args = fillers.pop(0)
                        fn(*args)
                        pumped[0] += 1

                def pump_until(k):
                    while pumped[0] < k and fillers:
                        pump(1)

                # ---- prelude: just enough for pair0/ib0 -----------------
                gate_group(0)
                for ib in range(NNB):
                    k_group(0, ib)
                q_group(0, 0)

                # ordered: 2 tasks per pair0 block boundary, then pair-1 K/Q
                for ib in range(1, NNB):
                    fillers.append((gate_group, (ib,)))
                    fillers.append((q_group, (0, ib)))
                for ib in range(NNB):
                    fillers.append((k_group, (1, ib)))
                for ib in range(NNB):
                    fillers.append((q_group, (1, ib)))

                # pump slots per (pair, ib): spaced mi indices. Each pair's
                # ib0 carries its V-projection inline; pair-1 po slots sit in
                # the back half so the normalize chain (recip+broadcast) of
                # the previous block has landed.
                slots = {
                    (0, 0): [15],
                    (0, 1): [1, 4, 7, 10, 13],
                    (0, 2): [1, 4, 7, 10, 13],
                    (0, 3): [1, 4, 7, 10],
                    (1, 0): [],
                    (1, 1): [4, 5, 6, 7, 8, 9, 10, 11],
                    (1, 2): [4, 5, 6, 7, 8, 9, 10, 11],
                    (1, 3): [4, 5, 6, 7, 8, 9, 10, 11],
                }

                # ---- attention ------------------------------------------
                for pair in range(2):
                    for ib in range(NNB):
                        if pair == 0:
                            pump_until(2 * ib)   # gate(ib), q0(ib) emitted
                        elif ib == 0:
                            pump_until(14)       # all projections emitted
                        nq = slice(ib * NB, (ib + 1) * NB)
                        avp = [psav.tile([DH + 1, NB], F32, tag="av",
                                         name=f"avp{rep}_{pair}_{ib}_{h}")
                               for h in range(2)]
                        for mi in range(NMI):
                            if ib == 0:
                                v_group(pair, mi)
                            ms = slice(mi * P, (mi + 1) * P)
                            s = pss.tile([P, 2 * NB], F32, tag="s",
                                         name=f"s{rep}_{pair}_{ib}_{mi}")
                            es = espool.tile([P, 2 * NB], BF16, tag="es")
                            for half in range(2):
                                d = slice(half * DH, (half + 1) * DH)
                                nc.tensor.matmul(
                                    s[:, half * NB:(half + 1) * NB],
                                    KT[pair][d, ms], QT[pair][d, nq],
                                    start=True, stop=True)
                            nc.scalar.activation(es[:], s[:], EXP)
                            for half in range(2):
                                nc.tensor.matmul(
                                    avp[half][:], Vn[:, mi, 2 * pair + half, :],
                                    es[:, half * NB:(half + 1) * NB],
                                    start=(mi == 0), stop=(mi == NMI - 1))
                            if mi in slots[(pair, ib)]:
                                pump(1)
                        normalize(pair, ib, avp)
                        if pair == 1:
                            for nqi in range(ib * 4, ib * 4 + 4):
                                for co in range(2):
                                    fillers.append((po_group, (nqi, co)))
                pump(len(fillers))

    nc.compile()
    return nc


def _bf16(a):
    return np.ascontiguousarray(a).astype(ml_dtypes.bfloat16)


def make_in_maps(x, Wqkv, We, Wo):
    in_maps = []
    for c in range(8):
        b, g = divmod(c, 4)
        cols = slice(g * HPC * DH, (g + 1) * HPC * DH)
        in_maps.append({
            "xT": _bf16(x[b].T),
            "wq": _bf16(Wqkv[:, 0 * C:1 * C][:, cols]),
            "wk": _bf16(Wqkv[:, 1 * C:2 * C][:, cols]),
            "wv": _bf16(Wqkv[:, 2 * C:3 * C][:, cols]),
            "we": _bf16(We[:, g * HPC:(g + 1) * HPC]),
            "wo": _bf16(Wo[cols, :]),
            "ones64": np.ones((P, NMI * HPC), dtype=ml_dtypes.bfloat16),
        })
    return in_maps


def kernel(x, attention_mask, Wqkv, bqkv, We, be, Wo, bo):
    x = np.asarray(x, dtype=np.float32)
    Wqkv = np.asarray(Wqkv, dtype=np.float32)
    We = np.asarray(We, dtype=np.float32)
    Wo = np.asarray(Wo, dtype=np.float32)

    if "nc" not in _CACHE:
        _CACHE["nc"] = _build()
    nc = _CACHE["nc"]

    in_maps = make_in_maps(x, Wqkv, We, Wo)

    trace = bool(int(os.environ.get("KERNEL_TRACE", "0")))
    res = run_bass_kernel_spmd(nc, in_maps, core_ids=list(range(8)), trace=trace)
    _CACHE["last_result"] = res

    parts = [np.asarray(res.results[c]["outp"]).astype(np.float32)
             for c in range(8)]
    out = np.stack([parts[0] + parts[1] + parts[2] + parts[3],
                    parts[4] + parts[5] + parts[6] + parts[7]])
    out += np.asarray(bo, dtype=np.float32)
    return out.astype(np.float32)
